# revision 45
# baseline (speedup 1.0000x reference)
"""Bass/Trainium2 kernel for nn_BiencoderRanker: pairwise cosine similarity.

scores[n, m] = <pred_n, cand_m> / (|pred_n| * |cand_m|)
  fp_pred: (1024, 4096) fp32, fp_cand: (16384, 4096) fp32 -> scores (1024, 16384) fp32

Sharding: fp_cand split along M across 8 cores (2048 rows each); fp_pred
replicated. Each core computes its (1024, 2048) tile; host concatenates.

Host marshalling: inputs are cast fp32->bf16 (RNE) once on host; the device
computes in bf16 with fp32 accumulation. Numerically this matches rounding
on-chip (which a cast-DMA would do anyway) and halves HBM read traffic.

Per-core dataflow (two independent paths joining at the output scaling):
  matmul path:  xbar DMA transpose-loads straight from DRAM -> K-on-partition
                aT / bT bf16 tiles; psum[n128, m512] += aT.T @ bT over 32
                K-chunks (TensorE); raw ACT drain (no deps on norms).
  norm path:    natural-layout bf16 row-chunk loads (SWDGE) -> square +
                row-sum (split between ACT activation-accum and DVE
                tensor_tensor_reduce) -> sqrt -> reciprocal = 1/|row|;
                1/|cand| round-trips through DRAM onto the free axis.
  join:         out-tile *= 1/|pred_n| (per-partition) then *= broadcast
                1/|cand_m| row (DVE), then store.

The DMA stream is explicitly phase-ordered (add_dep_helper edges) into
copy-batches and transpose-batches: every xbar-mode C<->T transition
serializes the whole DMA pipeline, so alternating them per-chunk is ruinous.
"""

import numpy as np
import ml_dtypes

import concourse.bacc as bacc
import concourse.mybir as mybir
import concourse.tile as tile
from concourse.bass_utils import run_bass_kernel_spmd
from concourse.tile_rust import add_dep_helper

P = 128
N = 1024  # fp_pred rows
K = 4096  # feature dim
M_FULL = 16384  # fp_cand rows
N_CORES = 8
M = M_FULL // N_CORES  # cand rows per core
NB = N // P  # 8 pred row-chunks
MBLK = 4  # m-blocks per core (512 cand rows each)
MC = 4  # 128-row chunks per m-block
KC = K // P  # 32 contraction chunks
FREE = 512  # matmul moving free dim / psum bank width

F32 = mybir.dt.float32
BF16 = mybir.dt.bfloat16
AF = mybir.ActivationFunctionType

_compiled = None


def _build(repeats=1, phase_edges=True):
    nc = bacc.Bacc(None, target_bir_lowering=False)
    pred = nc.dram_tensor("fp_pred_bf", (N, K), BF16, kind="ExternalInput")
    cand = nc.dram_tensor("fp_cand_bf", (M, K), BF16, kind="ExternalInput")
    out = nc.dram_tensor("scores", (N, M), F32, kind="ExternalOutput")

    # DMA phase plan: alternating transpose-batches and copy-batches. An
    # instruction's phase controls only its DMA ordering (via dep edges);
    # emission order (hence data-dependency tracking) is independent.
    PHASE_MODES = ["T", "C", "T", "C", "T", "C", "T", "C"] * repeats
    phases = [(m, []) for m in PHASE_MODES]
    cur_phase = [0]

    def set_phase(i):
        cur_phase[0] = i

    def ph(inst, phase=None):
        i = cur_phase[0] if phase is None else phase
        phases[i][1].append(inst.ins if hasattr(inst, "ins") else inst)
        return inst

    with tile.TileContext(nc) as tc:
        with (
            tc.tile_pool(name="dram", bufs=1, space="DRAM") as dram_pool,
            tc.tile_pool(name="at_pool", bufs=1) as at_pool,
            tc.tile_pool(name="bt_pool", bufs=2) as bt_pool,
            tc.tile_pool(name="stage", bufs=3) as stage_pool,
            tc.tile_pool(name="sq", bufs=2) as sq_pool,
            tc.tile_pool(name="norm", bufs=8) as norm_pool,
            tc.tile_pool(name="invs", bufs=2) as inv_pool,
            tc.tile_pool(name="outp", bufs=14) as out_pool,
            tc.tile_pool(name="psum", bufs=4, space="PSUM") as psum_pool,
        ):
            # staging to move 1/|cand| from the partition axis to the free axis
            invb_dram = dram_pool.tile([M], F32, name="invb_rt")
            # aT[k_in_chunk, nb, kc, n_in_chunk] = pred_bf[nb*128 + n, kc*128 + k]
            aT = at_pool.tile([P, NB, KC, P], BF16, name="aT")
            inv_a = norm_pool.tile([P, NB], F32, bufs=1, name="inv_a")
            bts = {}

            def bt_tile(mb):
                if mb not in bts:
                    # bT[k_in_chunk, mc, kc, m_in_chunk]
                    bts[mb] = bt_pool.tile(
                        [P, MC, KC, P], BF16, tag="bt", name=f"bT{mb}"
                    )
                return bts[mb]

            def t_a(nb):
                ph(nc.sync.dma_start_transpose(aT[:, nb], pred[nb * P : (nb + 1) * P, :]))

            def t_b(mb, mc):
                r0 = (mb * MC + mc) * P
                ph(nc.sync.dma_start_transpose(bt_tile(mb)[:, mc], cand[r0 : r0 + P, :]))

            # ---- norm path ----
            def norm_chunk(dram_rows, inv_dst, idx, use_dve, phase=None):
                """inv_dst [P, 1] <- 1/|row| for 128 bf16 rows."""
                nat = stage_pool.tile([P, K], BF16, tag="stage", name=f"nat{idx}")
                ph(nc.gpsimd.dma_start(nat[:], dram_rows), phase=phase)
                sq = sq_pool.tile([P, K], BF16, tag="sq", name=f"sq{idx}")
                ssq = norm_pool.tile([P, 1], F32, tag="norm", name=f"ssq{idx}")
                # NOTE: DVE tensor_tensor_reduce (fused) fails at runtime on
                # this hw/ucode. Split square-reduce across engines instead:
                # ACT one-op (Square+accum) for half the chunks, DVE two-op
                # (tensor_tensor square at 2x, then tensor_reduce) for the
                # rest, so neither engine paces the norm pipeline.
                if use_dve:
                    nc.vector.tensor_tensor(
                        sq[:], nat[:], nat[:], mybir.AluOpType.mult
                    )
                    nc.vector.tensor_reduce(
                        ssq[:], sq[:], mybir.AxisListType.X, mybir.AluOpType.add
                    )
                else:
                    nc.scalar.activation(sq[:], nat[:], AF.Square, accum_out=ssq[:])
                nrm = norm_pool.tile([P, 1], F32, tag="norm", name=f"nrm{idx}")
                nc.scalar.activation(nrm[:], ssq[:], AF.Sqrt)
                nc.vector.reciprocal(inv_dst, nrm[:])

            def norm_a(nb, phase=None):
                norm_chunk(
                    pred[nb * P : (nb + 1) * P, :],
                    inv_a[:, nb : nb + 1],
                    f"a{nb}",
                    nb % 2 == 0,
                    phase=phase,
                )

            invbs = {}

            def norm_b(mb, mc):
                if mb not in invbs:
                    invbs[mb] = inv_pool.tile([P, MC], F32, tag="invb", name=f"invb{mb}")
                r0 = (mb * MC + mc) * P
                norm_chunk(
                    cand[r0 : r0 + P, :],
                    invbs[mb][:, mc : mc + 1],
                    f"b{mb}_{mc}",
                    mc % 2 == 0,
                )

            def invb_roundtrip(mb):
                # scatter [P, MC] -> invb_rt[mb*512 + mc*128 + p], reload as a
                # row, broadcast across partitions.
                ph(
                    nc.scalar.dma_start(
                        invb_dram[mb * FREE : (mb + 1) * FREE].rearrange(
                            "(mc p) -> p mc", p=P
                        ),
                        invbs[mb][:],
                    )
                )
                row = inv_pool.tile([1, FREE], F32, tag="invrow", name=f"invrow{mb}")
                ph(
                    nc.scalar.dma_start(
                        row[:], invb_dram[None, mb * FREE : (mb + 1) * FREE]
                    )
                )
                bcast = inv_pool.tile([P, FREE], F32, tag="invbc", name=f"invbc{mb}")
                ph(nc.gpsimd.partition_broadcast(bcast[:], row[:]))
                return bcast

            # ---- matmul path ----
            def mm_block(mb, inv_bcast, narrow_first=False):
                bT = bts[mb]
                store_insts = []
                for nb in range(NB):
                    ot = out_pool.tile([P, FREE], F32, tag="out", name=f"ot{mb}_{nb}")
                    if nb == 0 and narrow_first:
                        # Startup ramp: 4 narrow [128,128] groups, each gated
                        # on a single bT chunk transpose, so the PE starts
                        # after the first 2 transposes instead of all 5.
                        for mc in range(MC):
                            psn = psum_pool.tile(
                                [P, P], F32, tag="psn", name=f"psn{mb}_{mc}"
                            )
                            for kc in range(KC):
                                nc.tensor.matmul(
                                    psn[:],
                                    aT[:, nb, kc, :],
                                    bT[:, mc, kc, :],
                                    start=(kc == 0),
                                    stop=(kc == KC - 1),
                                )
                            nc.scalar.activation(
                                ot[:, mc * P : (mc + 1) * P], psn[:], AF.Copy
                            )
                    else:
                        ps = psum_pool.tile(
                            [P, FREE], F32, tag="ps", name=f"ps{mb}_{nb}"
                        )
                        for kc in range(KC):
                            nc.tensor.matmul(
                                ps[:],
                                aT[:, nb, kc, :],
                                bT[:, :, kc, :],
                                start=(kc == 0),
                                stop=(kc == KC - 1),
                            )
                        nc.scalar.activation(ot[:], ps[:], AF.Copy)  # raw drain
                    nc.vector.tensor_scalar_mul(ot[:], ot[:], inv_a[:, nb : nb + 1])
                    nc.vector.tensor_tensor(
                        ot[:], ot[:], inv_bcast[:], mybir.AluOpType.mult
                    )
                    store_insts.append(
                        nc.scalar.dma_start(
                            out[nb * P : (nb + 1) * P, mb * FREE : (mb + 1) * FREE],
                            ot[:],
                        )
                    )
                return store_insts

            # ---- emission (program order = dependency order) ----
            for rep in range(repeats):
                B = rep * 8  # phase base (repeats>1 only for benchmarking)
                bts.clear()
                invbs.clear()

                set_phase(B + 0)  # T — one big T-batch: a-tiles, bT0, bT1,
                # ordered so each tile lands just before the PE needs it
                t_a(0)
                for mc in range(MC):
                    t_b(0, mc)
                for nb in range(1, 5):
                    t_a(nb)
                for mc in range(MC):
                    t_b(1, mc)
                for nb in range(5, NB):
                    t_a(nb)

                set_phase(B + 3)  # C
                for mc in range(MC):
                    norm_b(0, mc)
                for nb in range(4):
                    norm_a(nb)
                for mc in range(MC):
                    norm_b(1, mc)
                # a4-a7 chains are only needed for the later mb0 scales: emit
                # them last in this phase's slot-rotation order (before their
                # readers in mm_block(0)), but schedule the loads in P5.
                for nb in range(4, NB):
                    norm_a(nb, phase=B + 5)
                invbc0 = invb_roundtrip(0)
                stores0 = mm_block(0, invbc0, narrow_first=True)
                for s in stores0[:4]:
                    ph(s, phase=B + 3)
                for s in stores0[4:]:
                    ph(s, phase=B + 5)

                set_phase(B + 4)  # T
                for mc in range(MC):
                    t_b(2, mc)

                set_phase(B + 5)  # C
                for mc in range(MC):
                    norm_b(2, mc)
                invbc1 = invb_roundtrip(1)
                stores1 = mm_block(1, invbc1)
                for s in stores1[:4]:
                    ph(s, phase=B + 5)
                for s in stores1[4:]:
                    ph(s, phase=B + 7)

                set_phase(B + 6)  # T
                for mc in range(MC):
                    t_b(3, mc)

                set_phase(B + 7)  # C
                for mc in range(MC):
                    norm_b(3, mc)
                invbc2 = invb_roundtrip(2)
                stores2 = mm_block(2, invbc2)
                for s in stores2:
                    ph(s)
                invbc3 = invb_roundtrip(3)
                stores3 = mm_block(3, invbc3)
                for s in stores3:
                    ph(s)

            # phase-order edges: every DMA in phase i waits on every DMA in
            # the previous non-empty phase (C<->T xbar transitions batched)
            if phase_edges:
                nonempty = [pl for _, pl in phases if pl]
                for i in range(1, len(nonempty)):
                    for inst in nonempty[i]:
                        for prev in nonempty[i - 1]:
                            add_dep_helper(inst, prev, True, "dma xbar phase order")
    nc.compile()
    return nc


def _get_compiled():
    global _compiled
    if _compiled is None:
        _compiled = _build()
    return _compiled


def kernel(fp_pred: np.ndarray, fp_cand: np.ndarray) -> np.ndarray:
    fp_pred = np.asarray(fp_pred, dtype=np.float32)
    fp_cand = np.asarray(fp_cand, dtype=np.float32)
    assert fp_pred.shape == (N, K) and fp_cand.shape == (M_FULL, K)

    pred_bf = fp_pred.astype(ml_dtypes.bfloat16)
    cand_bf = fp_cand.astype(ml_dtypes.bfloat16)

    nc = _get_compiled()
    in_maps = [
        {
            "fp_pred_bf": pred_bf,
            "fp_cand_bf": np.ascontiguousarray(cand_bf[i * M : (i + 1) * M]),
        }
        for i in range(N_CORES)
    ]
    res = run_bass_kernel_spmd(nc, in_maps, core_ids=list(range(N_CORES)))
    return np.concatenate([res.results[i]["scores"] for i in range(N_CORES)], axis=1)
